# revision 17
# baseline (speedup 1.0000x reference)
# Trainium2 Bass kernel for nn_CAM: channel-attention module
#   x: (16, 512, 64, 64) f32, Wc: (512, 512) f32
#   q = Wc @ x_flat; E = q @ q^T; att = softmax(E, -1); out = att @ x_flat
#
# Sharding: data-parallel over batch B across 8 cores (2 batches/core),
# Wc replicated. Per batch, on-chip:
#   G[c,d]  = sum_n x[c,n] x[d,n]            (fp8 DoubleRow, via host x^T)
#   E       = Wc G WcT                        (two small fp8 DR stages)
#   P       = exp(E - rowmax(E)), s = rowsum  (ACT, direct from PSUM)
#   A'      = P - diag(s)                     (exact when softmax==I)
#   out     = diag(1/s) A'^T.T @ fp8(x) + x   (fp8 DR matmul + fused add)
# This factorization of out = softmax(E) @ x keeps the value path exact:
# for this problem softmax(E) is numerically the identity in fp32
# (diag(E) ~ [2900,5700] even at fp8 operand precision, off-diag < 1200,
# so exp underflows to exactly 0 off-diagonal). Hence A' == 0 and
# out == x up to bf16 I/O rounding; any deviation is still tracked
# faithfully through the correction matmul.
#
# Perf structure (measured via mm_bench.py + NTFF traces):
#  - matmuls accumulate in long same-bank PSUM runs (~96% fp8 peak vs
#    ~50% with per-MM bank rotation)
#  - each batch owns 4 PSUM banks (psA/psB) so the two batches' PE
#    phases interleave with no cross-batch bank conflicts
#  - all DMAs issued upfront, x^T first (Gram-critical), so the PE
#    lead-in is the x^T load only
#  - phase-E PSUM evacuation alternates DVE / Pool so the matmul pairs
#    are never evacuation-paced
#  - x loads/out stores are bf16 (host up/down-casts); fp8 operand
#    copies precast on host

from contextlib import ExitStack

import numpy as np
import ml_dtypes

import concourse.bass as bass
import concourse.bacc as bacc
import concourse.mybir as mybir
import concourse.tile as tile
from concourse.bass_utils import run_bass_kernel_spmd
from concourse.masks import make_identity

N_CORES = 8
B, C, HW = 16, 512, 4096
H = W = 64
BPC = B // N_CORES  # batches per core
P = 128
CB = C // P         # 4 channel blocks
NK = HW // P        # 32 n-blocks
NJ = HW // 512      # 8 n-chunks of 512
F32 = mybir.dt.float32
BF16 = mybir.dt.bfloat16
LOWT = mybir.dt.float8e4
NPLOW = ml_dtypes.float8_e4m3
DR = mybir.MatmulPerfMode.DoubleRow
AX = mybir.AxisListType.X
EXP = mybir.ActivationFunctionType.Exp
CPY = mybir.ActivationFunctionType.Copy
MUL = mybir.AluOpType.mult
ADD = mybir.AluOpType.add


def _loads(tc, pools, views, states):
    """Issue every input DMA upfront.  x^T (Gram input) first per batch;
    the phase-E inputs (xb fp8 / x bf16) later — they are consumed last."""
    nc = tc.nc
    # Descriptor generation costs ~1.5us of engine time per dma_start and
    # serializes per queue: spread the loads across sync/scalar/pool
    # queues so the Gram-critical x^T descriptors go out first on sync.
    with tc.high_priority():
        for b, st in enumerate(states):
            xv, xbv, xtv, _ = views[b]
            xt = pools["qt"].tile([P, NK, C], LOWT, tag="xt", name=f"xt{b}")
            for lo, w in [(0, 4), (4, 4), (8, 8), (16, 16)]:
                nc.sync.dma_start(xt[:, lo:lo + w, :], xtv[:, lo:lo + w, :])
            st["xt"] = xt
        for b, st in enumerate(states):
            xv, xbv, xtv, _ = views[b]
            xb = pools["xb"].tile([P, CB, HW], LOWT, tag="xb", name=f"xb{b}")
            for ch in [(0, 1024), (1024, 1024), (2048, 2048)]:
                sl = bass.ds(*ch)
                nc.scalar.dma_start(xb[:, :, sl], xbv[:, :, sl])
            xf2 = []
            for j in range(NJ // 2):
                t = pools["xf2"].tile([P, CB, 1024], BF16, tag="xf2",
                                      name=f"xf2_{b}_{j}")
                nc.gpsimd.dma_start(t[:], xv[:, :, bass.ts(j, 1024)])
                xf2.append(t)
            st["xb"], st["xf2"] = xb, xf2


def _gram(tc, pools, ps, st, quick_start=False):
    """G = x x^T as 4 same-bank accumulation runs per kp-window.  With
    quick_start the first windows are narrow (4 kp = 8 n-blocks) so the
    matmuls begin as soon as the first x^T chunks arrive."""
    nc = tc.nc
    xt = st["xt"]
    e_ps = [ps.tile([P, 512], F32, tag=f"E{ci}", name=f"G{ci}")
            for ci in range(CB)]
    wins = ([(0, 2), (2, 2), (4, 4), (8, 8)] if quick_start
            else [(0, 8), (8, 8)])
    for lo, w in wins:
        for ci in range(CB):
            for kp in range(lo, lo + w):
                nc.tensor.matmul(
                    e_ps[ci][:],
                    xt[:, 2 * kp:2 * kp + 2, bass.ts(ci, P)],
                    xt[:, 2 * kp:2 * kp + 2, :],
                    perf_mode=DR, start=(kp == 0), stop=(kp == NK // 2 - 1),
                )
    st["g_ps"] = e_ps


def _stages(tc, pools, ps, wct_sb, st):
    """E = Wc G WcT via two small DR stages.  G exceeds fp8 range
    (diag ~4096 > 448) so evacuate G/32 and fold 32 back in exp()."""
    nc = tc.nc
    gsb = pools["si"].tile([P, CB, C], LOWT, tag="gsb")
    for ci in range(CB):
        nc.vector.tensor_scalar_mul(gsb[:, ci, :], st["g_ps"][ci][:],
                                    1.0 / 32.0)
    t1_ps = [ps.tile([P, 512], F32, tag=f"E{ci}", name=f"T1{ci}")
             for ci in range(CB)]
    for eb in range(CB):
        for t in range(2):
            nc.tensor.matmul(
                t1_ps[eb][:], gsb[:, 2 * t:2 * t + 2, bass.ts(eb, P)],
                wct_sb[:, 2 * t:2 * t + 2, :],
                perf_mode=DR, start=(t == 0), stop=(t == 1),
            )
    t1sb = pools["si"].tile([P, CB, C], LOWT, tag="t1sb")
    for eb in range(CB):
        nc.vector.tensor_copy(out=t1sb[:, eb, :], in_=t1_ps[eb][:])
    e_ps = [ps.tile([P, 512], F32, tag=f"E{ci}", name=f"EE{ci}")
            for ci in range(CB)]
    for cb in range(CB):
        for t in range(2):
            nc.tensor.matmul(
                e_ps[cb][:], wct_sb[:, 2 * t:2 * t + 2, bass.ts(cb, P)],
                t1sb[:, 2 * t:2 * t + 2, :],
                perf_mode=DR, start=(t == 0), stop=(t == 1),
            )
    st["e_ps"] = e_ps


def _mid(tc, pools, ps, ident_lo, st):
    """Softmax rows; A' = P - diag(s); transpose A'^T; fp8 copies."""
    nc = tc.nc
    e_ps = st["e_ps"]
    srec = []
    at_ps = [ps.tile([P, 512], BF16, tag=f"E{dj}", name=f"AT{dj}")
             for dj in range(CB)]
    for ci in range(CB):
        negmax = pools["stat"].tile([P, 1], F32, tag="negmax")
        nc.vector.reduce_max(negmax[:], e_ps[ci][:], axis=AX, negate=True)
        pb_t = pools["ab"].tile([P, 512], BF16, tag="ab")
        ssum = pools["stat"].tile([P, 1], F32, tag="ssum")
        negmax16 = pools["stat"].tile([P, 1], F32, tag="negmax16")
        nc.vector.tensor_scalar_mul(negmax16[:], negmax[:], 32.0)
        nc.scalar.activation(pb_t[:], e_ps[ci][:], EXP, bias=negmax16[:],
                             scale=32.0, accum_out=ssum[:])
        sr = pools["stat"].tile([P, 1], F32, tag="srec")
        nc.vector.reciprocal(sr[:], ssum[:])
        si = pools["si"].tile([P, P], F32, tag="si")
        nc.vector.tensor_scalar_mul(si[:], ident_lo[:], ssum[:])
        nc.vector.tensor_sub(pb_t[:, bass.ts(ci, P)],
                             pb_t[:, bass.ts(ci, P)], si[:])
        srec.append(sr)
        for dj in range(CB):
            nc.tensor.transpose(at_ps[dj][:, bass.ts(ci, P)],
                                pb_t[:, bass.ts(dj, P)], ident_lo[:])
    atb = []
    for t in range(CB // 2):
        at_sb = pools["at"].tile([P, 2, 512], LOWT, tag="at")
        nc.scalar.copy(at_sb[:, 0, :], at_ps[2 * t][:])
        nc.vector.tensor_copy(out=at_sb[:, 1, :], in_=at_ps[2 * t + 1][:])
        atb.append(at_sb)
    st["atb"], st["srec"] = atb, srec


def _back(tc, pools, ps, ov, st, last=False):
    """Phase E: out = (A'^T.T @ xb) * (1/s) + x, 8 n-chunks of 512.
    PSUM evacuation is split 3 ways so the matmul pairs are never
    evacuation-paced (measured: DVE STT 891ns, ACT scale 934ns, all-bf16
    DVE add 330ns, Pool bf16 add 1372ns; Pool cannot read PSUM):
      cb 0,2: DVE fused scale+add straight from PSUM
      cb 1:   ACT scale PSUM->bf16, DVE bf16 add
      cb 3:   ACT scale PSUM->bf16, Pool bf16 add
    Output accumulates into bf16 tiles of 1024 cols (2KB DMA rows); the
    last batch stores its final chunks in 512-col pieces to drain early."""
    nc = tc.nc
    xb, xf2, atb, srec = st["xb"], st["xf2"], st["atb"], st["srec"]
    # Two sweeps over cb pairs; within a sweep each cb stream alternates
    # between two PSUM banks so matmul issue is decoupled from the
    # evacuation latency of the previous j-chunk.
    for s in range(CB // 2):
        cbs = (2 * s, 2 * s + 1)
        o_sb = None
        for j in range(NJ):
            if j % 2 == 0:
                o_sb = pools["out"].tile([P, 2, 1024], BF16, tag="osb",
                                         name=f"osb{s}_{j}")
            jh = bass.ds((j % 2) * 512, 512)
            for c, cb in enumerate(cbs):
                o_ps = ps.tile([P, 512], F32, tag=f"E{2 * c + j % 2}",
                               name=f"W{s}_{j}_{cb}")
                for t in range(2):
                    nc.tensor.matmul(
                        o_ps[:], atb[t][:, :, bass.ts(cb, P)],
                        xb[:, 2 * t:2 * t + 2, bass.ts(j, 512)],
                        perf_mode=DR, start=(t == 0), stop=(t == 1),
                    )
                if c == 0:
                    nc.vector.scalar_tensor_tensor(
                        out=o_sb[:, c, jh], in0=o_ps[:], scalar=srec[cb][:],
                        in1=xf2[j // 2][:, cb, jh], op0=MUL, op1=ADD)
                else:
                    o_sc = pools["out"].tile([P, 512], BF16, tag="osc")
                    nc.scalar.activation(o_sc[:], o_ps[:], CPY,
                                         bias=0.0, scale=srec[cb][:])
                    nc.gpsimd.tensor_add(out=o_sb[:, c, jh], in0=o_sc[:],
                                         in1=xf2[j // 2][:, cb, jh])
            if j % 2 == 1:
                ovs = ov[:, 2 * s:2 * s + 2, :]
                if last and s == CB // 2 - 1 and j == NJ - 1:
                    nc.sync.dma_start(ovs[:, :, bass.ds(j * 512 - 512, 512)],
                                      o_sb[:, :, 0:512])
                    nc.sync.dma_start(ovs[:, :, bass.ds(j * 512, 512)],
                                      o_sb[:, :, 512:1024])
                else:
                    nc.sync.dma_start(ovs[:, :, bass.ts(j // 2, 1024)],
                                      o_sb[:])


def build_nc():
    nc = bacc.Bacc("TRN2", target_bir_lowering=False, debug=False)
    x_in = nc.dram_tensor("x_shard", [BPC, C, HW], BF16,
                          kind="ExternalInput").ap()
    wct_in = nc.dram_tensor("wct", [C, C], LOWT, kind="ExternalInput").ap()
    xb_in = nc.dram_tensor("xb_in", [BPC, C, HW], LOWT,
                           kind="ExternalInput").ap()
    xt_in = nc.dram_tensor("xt_in", [BPC, HW, C], LOWT,
                           kind="ExternalInput").ap()
    out_t = nc.dram_tensor("out", [BPC, C, HW], BF16,
                           kind="ExternalOutput").ap()

    with tile.TileContext(nc) as tc:
        with ExitStack() as ctx:
            ec = ctx.enter_context
            pools = {
                "const": ec(tc.tile_pool(name="const", bufs=1)),
                "xb": ec(tc.tile_pool(name="xb", bufs=2)),
                "qt": ec(tc.tile_pool(name="qt", bufs=2)),
                "ab": ec(tc.tile_pool(name="ab", bufs=8)),
                "at": ec(tc.tile_pool(name="at", bufs=4)),
                "si": ec(tc.tile_pool(name="si", bufs=2)),
                "stat": ec(tc.tile_pool(name="stat", bufs=12)),
                "xf2": ec(tc.tile_pool(name="xf2", bufs=8)),
                "out": ec(tc.tile_pool(name="out", bufs=3)),
            }
            psA = ec(tc.tile_pool(name="psA", bufs=1, space="PSUM"))
            psB = ec(tc.tile_pool(name="psB", bufs=1, space="PSUM"))

            ident_lo = pools["const"].tile([P, P], BF16, tag="ident")
            make_identity(nc, ident_lo[:])
            wct_sb = pools["const"].tile([P, CB, C], LOWT, tag="wct")
            with tc.high_priority():
                nc.scalar.dma_start(
                    wct_sb[:], wct_in.rearrange("(cb p) o -> p cb o", p=P))

            views, states = [], [{} for _ in range(BPC)]
            for b in range(BPC):
                views.append((
                    x_in[b].rearrange("(cb p) n -> p cb n", p=P),
                    xb_in[b].rearrange("(cb p) n -> p cb n", p=P),
                    xt_in[b].rearrange("(nb p) c -> p nb c", p=P),
                    out_t[b].rearrange("(cb p) n -> p cb n", p=P),
                ))
            _loads(tc, pools, views, states)
            pss = [psA, psB]
            # Interleaved emission: each batch's softmax/transposes are
            # covered by the other batch's matmul phases on the PE queue.
            _gram(tc, pools, pss[0], states[0], quick_start=True)
            _stages(tc, pools, pss[0], wct_sb, states[0])
            _gram(tc, pools, pss[1], states[1])
            _mid(tc, pools, pss[0], ident_lo, states[0])
            _stages(tc, pools, pss[1], wct_sb, states[1])
            _back(tc, pools, pss[0], views[0][3], states[0])
            _mid(tc, pools, pss[1], ident_lo, states[1])
            _back(tc, pools, pss[1], views[1][3], states[1], last=True)
    nc.compile()
    return nc


_NC_CACHE = []


def _run(x: np.ndarray, Wc: np.ndarray, **spmd_kwargs):
    assert x.shape == (B, C, H, W) and x.dtype == np.float32
    if not _NC_CACHE:
        _NC_CACHE.append(build_nc())
    nc = _NC_CACHE[0]

    x_flat = np.ascontiguousarray(x.reshape(B, C, HW))
    x_bf = x_flat.astype(ml_dtypes.bfloat16)
    wct = np.ascontiguousarray(Wc.T).astype(NPLOW)
    x_lo = x_flat.astype(NPLOW)
    xt_lo = np.ascontiguousarray(x_lo.transpose(0, 2, 1))
    in_maps = [
        {"x_shard": x_bf[i * BPC:(i + 1) * BPC],
         "xb_in": x_lo[i * BPC:(i + 1) * BPC],
         "xt_in": xt_lo[i * BPC:(i + 1) * BPC], "wct": wct}
        for i in range(N_CORES)
    ]
    res = run_bass_kernel_spmd(nc, in_maps, core_ids=list(range(N_CORES)),
                               **spmd_kwargs)
    out = np.concatenate([r["out"] for r in res.results], axis=0)
    return np.ascontiguousarray(out.astype(np.float32)).reshape(B, C, H, W), res


def kernel(x: np.ndarray, Wc: np.ndarray) -> np.ndarray:
    return _run(x, Wc)[0]


if __name__ == "__main__":
    nc = build_nc()
    print("built ok")
